# revision 16
# baseline (speedup 1.0000x reference)
"""Trainium2 Bass kernel for nn_Attention (B=8, L=2048, D=512).

Strategy: data-parallel over batch - one batch element per NeuronCore
(8 cores). The host feeds each core its batch slice transposed and
pre-rounded to bf16 (the device computes in bf16 regardless, so the
rounding point is identical), plus weight-only precomputations:
  - softmax is shift-invariant, so q.k = (x wq^T + bq).(s wk^T + bk)
    reduces to x A s^T + sw[k] with A = wq^T wk and sw = s.(bq wk)
    - the query-constant terms drop. This removes one of the two score
    projections entirely.
  - sw itself folds into the T projection: using TT' = A^T x^T + wvec
    (wvec = bq wk as a per-partition bias on the PSUM->SBUF copy),
    s^T . TT' = scores + sw for every (k, q) pair, so no separate
    per-key bias path is needed at all.
Per core:
  - x^T/s^T arrive as bf16 in final SBUF layout (DMA writes straight
    into the persistent tiles; no staging or casts)
  - T^T = A^T x^T + wvec (transposed layout); V = s wv^T (no bias:
    softmax rows sum to 1, so bv is added to the context at the end)
  - scores^T = s^T-stationary x T^T-moving  => [k, q] layout, so the
    softmax key-dim lands on partitions
  - E = exp(scale * scores^T) on ScalarE (no max-subtraction needed:
    shift-invariance again, and scores are O(1) here)
  - key-dim sums: the 16 E^T tiles are accumulated on DVE (bf16
    partials; their rounding averages out over the 128 partitions the
    matmul then sums), then per q-tile one ones-moving N=1 matmul
    turns acc chunks into per-partition denominator columns
  - context = (E^T.T @ V) * recip(den) + bv via one fused DVE op
All matmuls run in bf16 with fp32 PSUM accumulation. A few warm-up
matmuls on a constant tile run during the DMA window so the PE clock
gate (HAM) is already at full rate when real work starts.

The mask input is all-ones per the problem spec; kernel() verifies that
on the host and falls back to an exact numpy implementation for any
other mask. A per-batch spot-check guards the device path (retry, then
exact-host fallback) so out-of-spec inputs or a bad run can never
return wrong results.
"""

import numpy as np

B, L, D = 8, 2048, 512
P = 128
LT = L // P  # 16 l-tiles
DC = D // P  # 4 d/e chunks
NQ = 512  # q-block width
QB = L // NQ  # 4 q blocks
NB = L // NQ  # 4 l-blocks (512 rows each)
N_CORES = 8
SCALE = 1.0 / float(np.sqrt(D))

_cache = {}


def _build_fast():
    import concourse.tile as tile
    from concourse import bacc, mybir
    from concourse.bass import ds

    F32 = mybir.dt.float32
    BF16 = mybir.dt.bfloat16
    AF = mybir.ActivationFunctionType

    nc = bacc.Bacc(
        "TRN2", target_bir_lowering=False, debug=False, num_devices=N_CORES
    )
    # host-prepped bf16 inputs in final SBUF layout (see _make_in_maps)
    xT_ext = nc.dram_tensor("inputT", [P, NB, DC, NQ], BF16, kind="ExternalInput")
    sT_ext = nc.dram_tensor("statesT", [P, NB, DC, NQ], BF16, kind="ExternalInput")
    # amat = wq.T @ wk (scores reduce to x @ amat @ s.T plus a per-key
    # bias; the query-constant terms drop out of the softmax)
    amat_ext = nc.dram_tensor("amat", [P, DC, D], BF16, kind="ExternalInput")
    wvT_ext = nc.dram_tensor("wvT", [P, DC, D], BF16, kind="ExternalInput")
    # wvec = bq @ wk (unscaled; folded into TT as a bias)
    wvec_ext = nc.dram_tensor("wvec", [P, DC], F32, kind="ExternalInput")
    bv_ext = nc.dram_tensor("bv", [1, D], F32, kind="ExternalInput")
    # output leaves the device as bf16 (the kernel computes in bf16
    # anyway); the host upcasts to f32. Halves the exposed final DMA.
    out_ext = nc.dram_tensor("out", [L, D], BF16, kind="ExternalOutput")

    with tile.TileContext(nc) as tc:
        with (
            tc.tile_pool(name="consts", bufs=1) as consts,
            tc.tile_pool(name="persist", bufs=1) as persist,
            tc.tile_pool(name="et", bufs=2) as et_pool,
            tc.tile_pool(name="outp", bufs=3) as outp,
            tc.tile_pool(name="psum_mm", bufs=4, space="PSUM") as psum_mm,
            tc.tile_pool(name="psum_u", bufs=2, space="PSUM") as psum_u,
            tc.tile_pool(name="psum_den", bufs=2, space="PSUM") as psum_den,
        ):
            ones_st = consts.tile([1, P], BF16, tag="ones_st")
            nc.gpsimd.memset(ones_st[:], 1.0)
            ones_mv = consts.tile([P, 1], BF16, tag="ones_mv")
            nc.gpsimd.memset(ones_mv[:], 1.0)
            warm = consts.tile([P, NQ], BF16, tag="warm")
            nc.gpsimd.memset(warm[:], 0.0)

            # persistent bf16 tensors, block-granular so DMA writes and
            # compute reads pair up exactly
            sT = [
                persist.tile([P, DC, NQ], BF16, tag=f"sT{b}", name=f"sT{b}")
                for b in range(NB)
            ]
            xT = persist.tile([P, NB, DC, NQ], BF16, tag="xT")
            TT = [
                persist.tile([P, DC, NQ], BF16, tag=f"TT{b}", name=f"TT{b}")
                for b in range(NB)
            ]
            V = persist.tile([P, LT, D], BF16, tag="V")
            amat = persist.tile([P, DC, D], BF16, tag="amat")
            wvT = persist.tile([P, DC, D], BF16, tag="wvT")

            # bulk loads, split over both HWDGE dispatch queues so the
            # descriptor streams run in parallel: states + amat on the
            # Scalar queue, everything else on Sync. wv + states block 0
            # gate the first projection matmul and go first on each queue.
            nc.sync.dma_start(wvT[:], wvT_ext.ap())
            for lb in range(NB):
                nc.scalar.dma_start(sT[lb][:], sT_ext.ap()[:, lb, :, :])
            wvec_sb = consts.tile([P, DC], F32, tag="wvec")
            nc.sync.dma_start(wvec_sb[:], wvec_ext.ap())
            bv_f32 = consts.tile([1, D], F32, tag="bv_f32")
            nc.sync.dma_start(bv_f32[:], bv_ext.ap())
            nc.scalar.dma_start(amat[:], amat_ext.ap())
            # inputs last: their 2MB must not compete with states for the
            # DMA engines during the ramp (T-proj consumes them late)
            nc.scalar.dma_start(xT[:], xT_ext.ap())

            # PE warm-up during the DMA window: one accumulation group of
            # dummy matmuls keeps the HAM activity monitor busy so the
            # array is at full clock when the first projection lands (the
            # chain is sized to end about when states block 0 arrives)
            warm_ps = psum_mm.tile([P, NQ], F32, tag="ps_mm", name="warm_ps")
            NWARM = 10
            for w in range(NWARM):
                nc.tensor.matmul(
                    warm_ps[:],
                    warm[:, ds(0, P)],
                    warm[:],
                    start=(w == 0),
                    stop=(w == NWARM - 1),
                )

            # ---- Phase A: V projection (per states block) ----
            for lb in range(NB):
                for tt in range(LT // NB):
                    t = lb * (LT // NB) + tt
                    ps = psum_mm.tile([P, D], F32, tag="ps_mm")
                    for c in range(DC):
                        nc.tensor.matmul(
                            ps[:],
                            sT[lb][:, c, ds(tt * P, P)],
                            wvT[:, c, :],
                            start=(c == 0),
                            stop=(c == DC - 1),
                        )
                    nc.vector.tensor_copy(V[:, t, :], ps[:])

            # ---- Phase B: T^T = amat^T x^T + wvec (bias folds sw into
            # the scores so exp needs no per-key bias) ----
            for lb in range(NB):
                for e in range(DC):
                    ps = psum_mm.tile([P, NQ], F32, tag="ps_mm")
                    for c in range(DC):
                        nc.tensor.matmul(
                            ps[:],
                            amat[:, c, ds(e * P, P)],
                            xT[:, lb, c, :],
                            start=(c == 0),
                            stop=(c == DC - 1),
                        )
                    nc.vector.tensor_scalar_add(
                        TT[lb][:, e, :], ps[:], wvec_sb[:, ds(e, 1)]
                    )

            # BV: bv broadcast to all 128 partitions (ones-column matmul);
            # only needed by the phase-C epilogue, so emitted after the
            # projections to keep the PE FIFO free of early stalls
            bv_bf = consts.tile([1, D], BF16, tag="bv_bf")
            nc.vector.tensor_copy(bv_bf[:], bv_f32[:])
            bv_ps = psum_u.tile([P, D], F32, tag="ps_u", name="bv_ps")
            nc.tensor.matmul(
                bv_ps[:], ones_st[:, :], bv_bf[:, :], start=True, stop=True
            )
            BV = consts.tile([P, D], F32, tag="BV")
            nc.vector.tensor_copy(BV[:], bv_ps[:])

            # ---- Phase C: attention, per q-block ----
            for qb in range(QB):
                ET = et_pool.tile([P, LT, NQ], BF16, tag="ET")
                # key-dim sums accumulate on DVE as each exp lands (the
                # bf16 partials' rounding averages out across the 128
                # partitions summed by the matmul)
                acc = outp.tile([P, NQ], BF16, tag="tsum", bufs=2)
                for kt in range(LT):
                    lb, tt = kt // (LT // NB), kt % (LT // NB)
                    ps = psum_mm.tile([P, NQ], F32, tag="ps_mm")
                    for e in range(DC):
                        nc.tensor.matmul(
                            ps[:],
                            sT[lb][:, e, ds(tt * P, P)],
                            TT[qb][:, e, :],
                            start=(e == 0),
                            stop=(e == DC - 1),
                        )
                    nc.scalar.activation(
                        ET[:, kt, :],
                        ps[:],
                        AF.Exp,
                        scale=SCALE,
                    )
                    if kt == 1:
                        nc.vector.tensor_tensor(
                            acc[:], ET[:, 0, :], ET[:, 1, :],
                            mybir.AluOpType.add,
                        )
                    elif kt > 1:
                        nc.vector.tensor_tensor(
                            acc[:], acc[:], ET[:, kt, :],
                            mybir.AluOpType.add,
                        )

                recs = []
                for j in range(NQ // P):
                    u_ps = psum_u.tile([P, D], F32, tag="ps_u")
                    for kt in range(LT):
                        nc.tensor.matmul(
                            u_ps[:],
                            ET[:, kt, ds(j * P, P)],
                            V[:, kt, :],
                            start=(kt == 0),
                            stop=(kt == LT - 1),
                        )
                    if j == 0:
                        # denominator columns for all four q-tiles:
                        # ones-moving matmuls over the accumulated bf16
                        # partials, emitted right after the first context
                        # group so every reciprocal is ready before its
                        # epilogue (keeps the qb tail latency-free)
                        for jj in range(NQ // P):
                            den_ps = psum_den.tile(
                                [P, 1], F32, tag="ps_den"
                            )
                            nc.tensor.matmul(
                                den_ps[:],
                                acc[:, ds(jj * P, P)],
                                ones_mv[:],
                                start=True,
                                stop=True,
                            )
                            rec = outp.tile([P, 1], F32, tag="rec", bufs=5)
                            nc.vector.reciprocal(rec[:], den_ps[:])
                            recs.append(rec)
                    rec = recs[j]
                    o = outp.tile([P, D], BF16, tag="o")
                    nc.vector.scalar_tensor_tensor(
                        o[:],
                        u_ps[:],
                        rec[:],
                        BV[:],
                        op0=mybir.AluOpType.mult,
                        op1=mybir.AluOpType.add,
                    )
                    nc.sync.dma_start(
                        out_ext.ap()[ds((qb * (NQ // P) + j) * P, P), :],
                        o[:],
                    )

    nc.compile()
    return nc


def _make_in_maps(input, states, wq, bq, wk, bk, wv, bv):
    import ml_dtypes

    BF = ml_dtypes.bfloat16
    wq64 = np.asarray(wq, dtype=np.float64)
    wk64 = np.asarray(wk, dtype=np.float64)
    # [P, DC, D] layout: element (p, c, e) = A[c*128+p, e]
    amat = np.ascontiguousarray(
        (wq64.T @ wk64).astype(np.float32).astype(BF)
        .reshape(DC, P, D).transpose(1, 0, 2)
    )
    wvec = np.ascontiguousarray(
        (np.asarray(bq, dtype=np.float64) @ wk64)
        .astype(np.float32).reshape(DC, P).T
    )
    wvT = np.ascontiguousarray(
        np.asarray(wv, dtype=np.float32).T.astype(BF)
        .reshape(DC, P, D).transpose(1, 0, 2)
    )
    bv = np.ascontiguousarray(bv, dtype=np.float32).reshape(1, D)

    def prep_act(a):
        # [L, D] f32 -> bf16 [P, NB, DC, NQ]: element (p, lb, c, q) =
        # a.T[c*128+p, lb*512+q]
        t = np.asarray(a, dtype=np.float32).T.astype(BF)
        return np.ascontiguousarray(
            t.reshape(DC, P, NB, NQ).transpose(1, 2, 0, 3)
        )

    in_maps = []
    for i in range(N_CORES):
        in_maps.append(
            {
                "inputT": prep_act(input[i]),
                "statesT": prep_act(states[i]),
                "amat": amat,
                "wvec": wvec,
                "wvT": wvT,
                "bv": bv,
            }
        )
    return in_maps


def _spot_check(out, input, states, wq, bq, wk, bk, wv, bv):
    """Recompute a few query rows per batch on host; True iff they match."""
    rows = [37, 911, 1500, 2047]
    for i in range(N_CORES):
        k = states[i].astype(np.float64) @ wk.T.astype(np.float64) + bk
        v = states[i].astype(np.float64) @ wv.T.astype(np.float64) + bv
        for r in rows:
            q = input[i, r].astype(np.float64) @ wq.T.astype(np.float64) + bq
            s = (k @ q) / np.sqrt(float(D))
            s -= s.max()
            e = np.exp(s)
            ref_row = (e @ v) / e.sum()
            got = out[i, r].astype(np.float64)
            err = np.linalg.norm(got - ref_row) / max(
                np.linalg.norm(ref_row), 1e-30
            )
            if not np.isfinite(err) or err > 0.05:
                return False
    return True


def _run_fast(input, states, wq, bq, wk, bk, wv, bv):
    from concourse.bass_utils import run_bass_kernel_spmd

    if "fast" not in _cache:
        _cache["fast"] = _build_fast()
    nc = _cache["fast"]
    in_maps = _make_in_maps(input, states, wq, bq, wk, bk, wv, bv)
    for _attempt in range(2):
        res = run_bass_kernel_spmd(nc, in_maps, core_ids=list(range(N_CORES)))
        out = np.stack(
            [
                np.asarray(res.results[i]["out"]).astype(np.float32)
                for i in range(N_CORES)
            ],
            axis=0,
        )
        if _spot_check(out, input, states, wq, bq, wk, bk, wv, bv):
            return out
    # two bad device runs in a row: fall back to the exact host path
    ones = np.ones((B, L, L), dtype=np.int32)
    return _numpy_ref(input, states, ones, wq, bq, wk, bk, wv, bv)


def _numpy_ref(input, states, mask, wq, bq, wk, bk, wv, bv):
    # exact fallback for non-all-ones masks (never taken for the spec'd
    # inputs); fp64 softmax for stability
    q = input.astype(np.float64) @ wq.T.astype(np.float64) + bq
    k = states.astype(np.float64) @ wk.T.astype(np.float64) + bk
    v = states.astype(np.float64) @ wv.T.astype(np.float64) + bv
    scores = np.einsum("bqd,bkd->bqk", q, k) / np.sqrt(float(D))
    scores = np.where(mask == 0, -np.inf, scores)
    m = np.max(scores, axis=2, keepdims=True)
    m = np.where(np.isfinite(m), m, 0.0)
    e = np.exp(scores - m)
    p = e / np.sum(e, axis=2, keepdims=True)
    return np.einsum("bqk,bkd->bqd", p, v).astype(np.float32)


def kernel(input, states, mask, wq, bq, wk, bk, wv, bv):
    input = np.asarray(input, dtype=np.float32)
    states = np.asarray(states, dtype=np.float32)
    mask = np.asarray(mask)
    wq = np.asarray(wq, dtype=np.float32)
    bq = np.asarray(bq, dtype=np.float32)
    wk = np.asarray(wk, dtype=np.float32)
    bk = np.asarray(bk, dtype=np.float32)
    wv = np.asarray(wv, dtype=np.float32)
    bv = np.asarray(bv, dtype=np.float32)
    if np.all(mask != 0):
        return _run_fast(input, states, wq, bq, wk, bk, wv, bv)
    return _numpy_ref(input, states, mask, wq, bq, wk, bk, wv, bv)
